# revision 35
# baseline (speedup 1.0000x reference)
"""ARD-RBF kernel matrix on 8 TRN2 NeuronCores.

Math (reference):
    alpha = softmax(alpha_raw^2)            (D,)
    var   = variance_raw^2                  scalar
    sq_ij = sum_d alpha_d (x1_id - x2_jd)^2
    out   = var * exp(-0.5 * sq)            (N, M) f32

Device formulation (rows of x1 sharded 8 ways; per core):
    out_ij = exp( (16*cross_ij)/16 - 0.5*ra_i + ln var ) * exp(-0.5*rb_j)
    cross  = x1 @ (alpha * x2)^T            fp16 matmul, f32 PSUM accum
b = 16*alpha*x2 is pre-scaled (fp16 subnormal avoidance); ACT's free
input scale (1/16) undoes it.

Pacing: ScalarE (ACT) is the only exp engine: 32 x [128,2048] Exp ops at
~2.0us = ~64us/core; PE (8 fp16 N=512 MMs/group) matches it at ~2.05us.

DMA model (measured): every [128, w] DMA costs one descriptor per
partition row; the rings drain ~30-45 desc/us each and all rings share
the 16 SDMA engines, which round-robin across QUEUES (not bytes).  So
descriptor count is the scarce resource, per-partition spans must be
>=4KB, and whichever queue holds a critical transfer must not share
the early window with bulk.  Consequences baked in:
  - three issue queues (gpsimd SW ring, sync HW ring, scalar ACT-HW
    ring) each lead with ONE critical piece: x2-g0-j01 / x1 / x2-g0-j23
    (x1 carries the f32 bias block bitcast-packed in its last 16 fp16
    cols, so there is no separate tiny-packet bias DMA);
  - the late bulk (erb, x2 g1-g3) is gated behind the criticals and
    ordered by need time; x2 g3 is injected mid-way into sync's output
    loop; the ACT queue issues no gated DMAs (each issue is ~0.65us
    of pacer time);
  - x2's device layout interleaves (k0,k1) per 512-col chunk so each
    piece is one contiguous >=4KB/partition DMA;
  - outputs alternate rings by group parity; the last 2 groups'
    chunks are partition-split across all three queues.
Output DRAM layout is block [NT, NG, P, JG]; the host re-tiles.

Startup: boot barrier ~7-10us (run-varying), first ring data +2.5us;
junk matmuls bridge PE to data arrival so the HAM p-state stays warm;
G0 and G1 run as 4 512-col chunks each so ACT streams while the PE
clock ramps.

ot/erb are fp16 (not bf16): 3x lower rounding error (rel ~2.7e-3).
"""

import math
import sys

import numpy as np

import ml_dtypes  # noqa: F401

if "/opt/trn_rl_repo" not in sys.path:
    sys.path.insert(0, "/opt/trn_rl_repo")

N, M, D = 8192, 8192, 256
NCORES = 8
NS = N // NCORES          # 1024 rows of x1 per core
P = 128                   # partitions
KT = D // P               # 2 k-tiles
NG = 4                    # x2 column groups
JG = M // NG              # 2048 cols per group
NJ = 512                  # matmul moving free dim (1 PSUM bank)
NT = NS // P              # 8 row tiles per core
NGRP = NG * NT            # 32 groups
XB = 2 * NT               # bias packed as 16 fp16 cols after x1

SCALE_B = 16.0            # b pre-scale; ACT applies 1/SCALE_B

_F16 = np.float16

_compiled = None

WARM = 10                 # junk warmup matmuls (N=512) pre-data


def _build():
    import concourse.bass as bass
    import concourse.mybir as mybir
    from concourse.env import get_walrus_max_sem_num
    from contextlib import ExitStack

    base = get_walrus_max_sem_num()
    bass.get_kernel_semaphore_range = lambda: range(base, base + 28)

    dt = mybir.dt
    nc = bass.Bass()

    x1d = nc.declare_dram_parameter("x1d", [P, KT * NS + XB], dt.float16, isOutput=False)
    x2d = nc.declare_dram_parameter("x2d", [P, NG * KT * JG], dt.float16, isOutput=False)
    rbd = nc.declare_dram_parameter("rbd", [P, M], dt.float16, isOutput=False)
    outd = nc.declare_dram_parameter("out", [NT, NG, P, JG], dt.float16, isOutput=True)

    exp_f = mybir.ActivationFunctionType.Exp
    njc = JG // NJ            # 4 matmul column chunks per group
    OTN = 8
    INV_SB = 1.0 / SCALE_B

    def units_of(G):          # mul granularity (tail groups split)
        if G >= NGRP - 2:
            h = JG // 2
            return [(0, h), (h, JG)]
        return [(0, JG)]

    def act_units(G):         # ACT op granularity
        if G <= 1:            # ramp: chunked so ACT streams while PE is cold
            return [(c * NJ, (c + 1) * NJ) for c in range(njc)]
        if G == NGRP - 1:
            h = JG // 2
            return [(0, h), (h, JG)]
        return [(0, JG)]

    # output DMA pieces: (G, unit_n, lo, hi, plo, phi, queue)
    # queue: 0 = gpsimd (SW ring), 1 = sync (HW ring), 2 = scalar (HW ring)
    pieces = []
    for G in range(NGRP):
        us = units_of(G)
        for n, (lo, hi) in enumerate(us):
            if G == NGRP - 2:
                pieces.append((G, n, lo, hi, 0, 64, 0))
                pieces.append((G, n, lo, hi, 64, 128, 1))
            elif G == NGRP - 1:
                q2 = 1 if n == 0 else 0
                pieces.append((G, n, lo, hi, 0, 64, 2))
                pieces.append((G, n, lo, hi, 64, 128, q2))
            else:
                pieces.append((G, n, lo, hi, 0, 128, G % 2))

    acs_after = {}
    c = 0
    for G in range(NGRP):
        c += len(act_units(G))
        acs_after[G] = c
    vcs_after = {}
    c = 0
    for G in range(NGRP):
        c += len(units_of(G))
        vcs_after[G] = c
    slot_cum = {}
    slot_total = {}
    for s in range(OTN):
        tot = 0
        for G in range(s, NGRP, OTN):
            tot += sum(1 for p in pieces if p[0] == G)
            slot_cum[(s, G)] = tot
        slot_total[s] = tot

    def gt(G):
        return divmod(G, NT)

    # PE increments pes per j-chunk for the ramp groups, per group after
    pes_after = lambda G: (njc * (G + 1) if G <= 1 else 2 * njc + G - 1)

    with ExitStack() as _ctx:
        ec = _ctx.enter_context
        x1s = ec(nc.sbuf_tensor("x1s", [P, KT * NS + XB], dt.float16))
        x2s = ec(nc.sbuf_tensor("x2s", [P, NG * KT * JG], dt.float16))
        erb = ec(nc.sbuf_tensor("erb", [P, M], dt.float16))
        ots = [ec(nc.sbuf_tensor(f"ot{i}", [P, JG], dt.float16)) for i in range(OTN)]
        wrm = ec(nc.sbuf_tensor("wrm", [P, P + NJ], dt.float16))  # uninit junk
        scr = ec(nc.sbuf_tensor("scr", [1, 32], dt.float32))     # table preload
        ps0 = ec(nc.psum_tensor("ps0", [P, JG], dt.float32))
        ps1 = ec(nc.psum_tensor("ps1", [P, JG], dt.float32))
        pss = [ps0, ps1]
        biav = x1s[:, KT * NS:KT * NS + XB].bitcast(dt.float32)  # [P, NT] f32

        s_x1 = ec(nc.semaphore("s_x1"))      # x1 + bias         (16)
        s_x2a = ec(nc.semaphore("s_x2a"))    # x2 g0 j0+j1       (16)
        s_x2b = ec(nc.semaphore("s_x2b"))    # x2 g0 j2+j3       (16)
        s_x2g1 = ec(nc.semaphore("s_x2g1"))  # x2 g1             (16)
        s_x2g2 = ec(nc.semaphore("s_x2g2"))  # x2 g2             (16)
        s_x2g3 = ec(nc.semaphore("s_x2g3"))  # x2 g3             (16)
        ebA = ec(nc.semaphore("ebA"))        # erb g0            (16)
        ebB = ec(nc.semaphore("ebB"))        # erb g1            (16)
        ebC = ec(nc.semaphore("ebC"))        # erb g2+g3         (16)
        wz = ec(nc.semaphore("wz"))          # wrm initialized
        pes = ec(nc.semaphore("pes"))
        acs = ec(nc.semaphore("acs"))
        vcs = ec(nc.semaphore("vcs"))
        dps = [ec(nc.semaphore(f"dp{i}")) for i in range(OTN)]
        block = ec(nc.Block())

        ebs_l = [ebA, ebB, ebC, ebC]
        ebs_n = [16, 16, 16, 16]

        def issue_piece(q, pc):
            (G, n, lo, hi, plo, phi, _) = pc
            g, t = gt(G)
            us = units_of(G)
            q.wait_ge(vcs, vcs_after[G] - len(us) + 1 + n)
            q.dma_start(
                outd[t, g, plo:phi, lo:hi],
                ots[G % OTN][plo:phi, lo:hi],
            ).then_inc(dps[G % OTN], 16)

        @block.gpsimd
        def _(gpsimd):
            # SW ring, in global need order: x2 g0 j0+j1 (critical), erb g0
            # (gates mul(G0) -> the G0 output chunk), then even outputs with
            # x2 g1 injected after G2's issue (needed by PE(G8))
            gpsimd.dma_start(x2s[:, 0:JG], x2d[:, 0:JG]).then_inc(s_x2a, 16)
            gpsimd.dma_start(erb[:, 0:JG], rbd[:, 0:JG]).then_inc(ebA, 16)
            gpsimd.dma_start(x2s[:, 2 * JG:4 * JG], x2d[:, 2 * JG:4 * JG]).then_inc(s_x2g1, 16)
            gpsimd.dma_start(x2s[:, 4 * JG:6 * JG], x2d[:, 4 * JG:6 * JG]).then_inc(s_x2g2, 16)
            for pc in pieces:
                if pc[6] == 0:
                    issue_piece(gpsimd, pc)

        @block.sync
        def _(sync):
            # HW ring: x1+bias (critical); late bulk gated behind the
            # criticals; odd outputs; x2 g3 injected mid-stream
            sync.dma_start(x1s[:, :], x1d[:, :]).then_inc(s_x1, 16)
            sync.wait_ge(s_x2a, 16)
            sync.wait_ge(s_x2b, 16)
            sync.dma_start(erb[:, JG:2 * JG], rbd[:, JG:2 * JG]).then_inc(ebB, 16)
            sync.dma_start(erb[:, 2 * JG:4 * JG], rbd[:, 2 * JG:4 * JG]).then_inc(ebC, 16)
            for pc in pieces:
                if pc[6] == 1:
                    issue_piece(sync, pc)
                    if pc[0] == 9:
                        sync.dma_start(x2s[:, 6 * JG:8 * JG],
                                       x2d[:, 6 * JG:8 * JG]).then_inc(s_x2g3, 16)
            for s in range(OTN):
                sync.wait_ge(dps[s], 16 * slot_total[s])

        @block.tensor
        def _(tensor):
            # junk matmuls keep PE busy (HAM warm) until data arrives
            tensor.wait_ge(wz, 1)
            for i in range(WARM):
                mm = tensor.matmul(ps1[:, 0:NJ], wrm[:, 0:P], wrm[:, P:P + NJ],
                                   start=True, stop=True)
                if i > 0:
                    inst = mm.ins
                    inst = inst[0] if isinstance(inst, (list, tuple)) else inst
                    inst.ldweights = False
            # ramp groups G0/G1: j-outer, (k0,k1) per 512-col chunk so ACT
            # drains per chunk while the PE clock is still cold
            for RG in range(2):
                ps = pss[RG % 2]
                for j in range(njc):
                    for k in range(KT):
                        if RG == 0 and j == 0 and k == 0:
                            tensor.wait_ge(s_x1, 16)
                            tensor.wait_ge(s_x2a, 16)
                        if RG == 0 and j == 2 and k == 0:
                            tensor.wait_ge(s_x2b, 16)
                        mm = tensor.matmul(
                            ps[:, j * NJ:(j + 1) * NJ],
                            x1s[:, k * NS + RG * P: k * NS + (RG + 1) * P],
                            x2s[:, j * (KT * NJ) + k * NJ:
                                   j * (KT * NJ) + (k + 1) * NJ],
                            start=(k == 0),
                            stop=(k == KT - 1),
                        )
                    mm.then_inc(pes)
            for G in range(2, NGRP):
                g, t = gt(G)
                if G == NT:
                    tensor.wait_ge(s_x2g1, 16)
                if G == 2 * NT:
                    tensor.wait_ge(s_x2g2, 16)
                if G == 3 * NT:
                    tensor.wait_ge(s_x2g3, 16)
                if G >= 2:
                    tensor.wait_ge(acs, acs_after[G - 2])  # psum half free
                ps = pss[G % 2]
                for k in range(KT):
                    for j in range(njc):
                        mm = tensor.matmul(
                            ps[:, j * NJ:(j + 1) * NJ],
                            x1s[:, k * NS + t * P: k * NS + (t + 1) * P],
                            x2s[:, g * (KT * JG) + j * (KT * NJ) + k * NJ:
                                   g * (KT * JG) + j * (KT * NJ) + (k + 1) * NJ],
                            start=(k == 0),
                            stop=(k == KT - 1),
                        )
                        if j > 0:
                            # same stationary weights as previous matmul:
                            # suppress the redundant LDWEIGHTS
                            inst = mm.ins
                            inst = inst[0] if isinstance(inst, (list, tuple)) else inst
                            inst.ldweights = False
                mm.then_inc(pes)

        @block.scalar
        def _(scalar):
            # touch Exp early (junk SBUF, no DMA dep) so ACT_TABLE_LOAD
            # overlaps engine boot + input DMAs
            scalar.activation(scr[0:1, 16:32], scr[0:1, 0:16], exp_f)
            # third ring (ACT HW DGE): only x2 g0 j2+j3 — the ACT queue
            # must stay clear of gated issue work
            scalar.dma_start(x2s[:, JG:2 * JG], x2d[:, JG:2 * JG]).then_inc(s_x2b, 16)
            scalar.wait_ge(s_x1, 16)   # bias rides the x1 DMA
            for G in range(NGRP):
                g, t = gt(G)
                if G >= OTN:
                    # just-in-time slot-reuse wait: tolerates maximal ring lag
                    sl = G % OTN
                    scalar.wait_ge(dps[sl], 16 * slot_cum[(sl, G - OTN)])
                for n, (lo, hi) in enumerate(act_units(G)):
                    if G <= 1:
                        scalar.wait_ge(pes, pes_after(G) - njc + n + 1)
                    elif n == 0:
                        scalar.wait_ge(pes, pes_after(G))
                    scalar.activation(
                        ots[G % OTN][:, lo:hi],
                        pss[G % 2][:, lo:hi],
                        exp_f,
                        bias=biav[:, t:t + 1],
                        scale=INV_SB,
                    ).then_inc(acs)
            # tail: G31's lower-partition pieces on the (now idle) ACT ring
            for pc in pieces:
                if pc[6] == 2:
                    issue_piece(scalar, pc)

        @block.vector
        def _(vector):
            # fill the junk-matmul operand with normal values: uninit SBUF
            # can hold fp16 NaN/denormal patterns that throttle the PE
            vector.memset(wrm[:, :], 1.0).then_inc(wz, 1)
            for G in range(NGRP):
                g, t = gt(G)
                if t == 0:
                    vector.wait_ge(ebs_l[g], ebs_n[g])
                us = units_of(G)
                aus = act_units(G)
                for (lo, hi) in us:
                    need = acs_after[G] - len(aus)
                    for (alo, ahi) in aus:
                        need += 1
                        if ahi >= hi:
                            break
                    vector.wait_ge(acs, need)
                    vector.tensor_mul(ots[G % OTN][:, lo:hi], ots[G % OTN][:, lo:hi],
                                      erb[:, g * JG + lo:g * JG + hi]).then_inc(vcs)

    return nc


def _prep(x1, x2, alpha_raw, variance_raw):
    x1 = np.ascontiguousarray(np.asarray(x1, dtype=np.float32))
    x2 = np.ascontiguousarray(np.asarray(x2, dtype=np.float32))
    ar = np.asarray(alpha_raw, dtype=np.float64).reshape(-1)
    vr = np.asarray(variance_raw, dtype=np.float64).reshape(-1)

    a2 = ar * ar
    e = np.exp(a2 - a2.max())
    alpha = e / e.sum()                                   # (D,) f64
    var = float(vr[0]) ** 2
    if var > 0.0:
        logvar, post = math.log(var), None
    else:
        logvar, post = 0.0, var

    b = (SCALE_B * alpha[None, :]) * x2.astype(np.float64)  # (M, D)
    x2tm = b.T.reshape(KT, P, M).astype(_F16)             # [k, p, col]
    # device layout: col = g*(KT*JG) + jc*(KT*NJ) + k*NJ + jj
    njc = JG // NJ
    x2c = np.ascontiguousarray(
        x2tm.reshape(KT, P, NG, njc, NJ)
        .transpose(1, 2, 3, 0, 4)
        .reshape(P, NG * KT * JG))
    x1tm = x1.T.reshape(KT, P, N).astype(_F16)            # [k, p, row]

    ra = (x1.astype(np.float64) ** 2) @ alpha             # (N,)
    rb = (x2.astype(np.float64) ** 2) @ alpha             # (M,)
    bia = (-0.5 * ra + logvar).astype(np.float32)         # (N,)
    rbrow = np.exp(-0.5 * rb).astype(_F16).reshape(1, M)
    rbd = np.ascontiguousarray(np.broadcast_to(rbrow, (P, M)))

    in_maps = []
    for c in range(NCORES):
        sl = slice(c * NS, (c + 1) * NS)
        bia2 = np.ascontiguousarray(
            bia[sl].reshape(NT, P).T.astype(np.float32))   # [p, t]
        bia16 = bia2.view(np.float16)                      # [p, 2*NT] raw bytes
        x1c = np.ascontiguousarray(
            np.concatenate([x1tm[0][:, sl], x1tm[1][:, sl], bia16], axis=1))
        in_maps.append({
            "x1d": x1c,
            "x2d": x2c,
            "rbd": rbd,
        })
    return in_maps, post


def _untile(blk):
    # [NT, NG, P, JG] -> [NS, M]
    a = np.asarray(blk).reshape(NT, NG, P, JG)
    return a.transpose(0, 2, 1, 3).reshape(NS, M)


def _run(in_maps, trace=False):
    global _compiled
    from concourse.bass_utils import run_bass_kernel_spmd

    if _compiled is None:
        _compiled = _build()
    return run_bass_kernel_spmd(
        _compiled, in_maps, core_ids=list(range(NCORES)), trace=trace
    )


def kernel(x1, x2, alpha_raw, variance_raw):
    in_maps, post = _prep(x1, x2, alpha_raw, variance_raw)
    res = _run(in_maps)
    full = np.concatenate(
        [_untile(res.results[c]["out"]).astype(np.float32) for c in range(NCORES)],
        axis=0)
    if post is not None:
        full = (full * post).astype(np.float32)
    return full


# revision 36
# speedup vs baseline: 1.1597x; 1.1597x over previous
"""ARD-RBF kernel matrix on 8 TRN2 NeuronCores.

Math (reference):
    alpha = softmax(alpha_raw^2)            (D,)
    var   = variance_raw^2                  scalar
    sq_ij = sum_d alpha_d (x1_id - x2_jd)^2
    out   = var * exp(-0.5 * sq)            (N, M) f32

Device formulation (rows of x1 sharded 8 ways; per core):
    out_ij = exp( (16*cross_ij)/16 - 0.5*ra_i + ln var ) * exp(-0.5*rb_j)
    cross  = x1 @ (alpha * x2)^T            fp16 matmul, f32 PSUM accum
b = 16*alpha*x2 is pre-scaled (fp16 subnormal avoidance); ACT's free
input scale (1/16) undoes it.

Pacing: ScalarE (ACT) is the only exp engine: 32 x [128,2048] Exp ops at
~2.0us = ~64us/core; PE (8 fp16 N=512 MMs/group) matches it at ~2.05us.

DMA model (measured): every [128, w] DMA costs one descriptor per
partition row; the rings drain ~30-45 desc/us each and all rings share
the 16 SDMA engines, which round-robin across QUEUES (not bytes).  So
descriptor count is the scarce resource, per-partition spans must be
>=4KB, and whichever queue holds a critical transfer must not share
the early window with bulk.  Consequences baked in:
  - three issue queues (gpsimd SW ring, sync HW ring, scalar ACT-HW
    ring) each lead with ONE critical piece: x2-g0-j01 / x1 / x2-g0-j23
    (x1 carries the f32 bias block bitcast-packed in its last 16 fp16
    cols, so there is no separate tiny-packet bias DMA);
  - the late bulk (erb, x2 g1-g3) is gated behind the criticals and
    ordered by need time; x2 g3 is injected mid-way into sync's output
    loop; the ACT queue issues no gated DMAs (each issue is ~0.65us
    of pacer time);
  - x2's device layout interleaves (k0,k1) per 512-col chunk so each
    piece is one contiguous >=4KB/partition DMA;
  - outputs alternate rings by group parity; the last 2 groups'
    chunks are partition-split across all three queues.
Output DRAM layout is block [NT, NG, P, JG]; the host re-tiles.

Startup: boot barrier ~7-10us (run-varying), first ring data +2.5us;
junk matmuls bridge PE to data arrival so the HAM p-state stays warm;
G0 and G1 run as 4 512-col chunks each so ACT streams while the PE
clock ramps.

ot/erb are fp16 (not bf16): 3x lower rounding error (rel ~2.7e-3).
"""

import math
import sys

import numpy as np

import ml_dtypes  # noqa: F401

if "/opt/trn_rl_repo" not in sys.path:
    sys.path.insert(0, "/opt/trn_rl_repo")

N, M, D = 8192, 8192, 256
NCORES = 8
NS = N // NCORES          # 1024 rows of x1 per core
P = 128                   # partitions
KT = D // P               # 2 k-tiles
NG = 4                    # x2 column groups
JG = M // NG              # 2048 cols per group
NJ = 512                  # matmul moving free dim (1 PSUM bank)
NT = NS // P              # 8 row tiles per core
NGRP = NG * NT            # 32 groups
XB = 2 * NT               # bias packed as 16 fp16 cols after x1

SCALE_B = 16.0            # b pre-scale; ACT applies 1/SCALE_B

_F16 = np.float16

_compiled = None

WARM = 10                 # junk warmup matmuls (N=512) pre-data


def _build():
    import concourse.bass as bass
    import concourse.mybir as mybir
    from concourse.env import get_walrus_max_sem_num
    from contextlib import ExitStack

    base = get_walrus_max_sem_num()
    bass.get_kernel_semaphore_range = lambda: range(base, base + 28)

    dt = mybir.dt
    nc = bass.Bass()

    x1d = nc.declare_dram_parameter("x1d", [P, KT * NS + XB], dt.float16, isOutput=False)
    x2d = nc.declare_dram_parameter("x2d", [P, NG * KT * JG], dt.float16, isOutput=False)
    rbd = nc.declare_dram_parameter("rbd", [P, M], dt.float16, isOutput=False)
    outd = nc.declare_dram_parameter("out", [NT, NG, P, JG], dt.float16, isOutput=True)

    exp_f = mybir.ActivationFunctionType.Exp
    njc = JG // NJ            # 4 matmul column chunks per group
    OTN = 8
    INV_SB = 1.0 / SCALE_B

    def units_of(G):          # mul granularity (tail groups split)
        if G >= NGRP - 2:
            h = JG // 2
            return [(0, h), (h, JG)]
        return [(0, JG)]

    def act_units(G):         # ACT op granularity
        if G <= 1:            # ramp: chunked so ACT streams while PE is cold
            return [(c * NJ, (c + 1) * NJ) for c in range(njc)]
        if G == NGRP - 1:
            h = JG // 2
            return [(0, h), (h, JG)]
        return [(0, JG)]

    # output DMA pieces: (G, unit_n, lo, hi, plo, phi, queue)
    # queue: 0 = gpsimd (SW ring), 1 = sync (HW ring), 2 = scalar (HW ring)
    pieces = []
    for G in range(NGRP):
        us = units_of(G)
        for n, (lo, hi) in enumerate(us):
            if G == NGRP - 2:
                pieces.append((G, n, lo, hi, 0, 64, 0))
                pieces.append((G, n, lo, hi, 64, 128, 1))
            elif G == NGRP - 1:
                q2 = 1 if n == 0 else 0
                pieces.append((G, n, lo, hi, 0, 64, 2))
                pieces.append((G, n, lo, hi, 64, 128, q2))
            else:
                pieces.append((G, n, lo, hi, 0, 128, G % 2))

    acs_after = {}
    c = 0
    for G in range(NGRP):
        c += len(act_units(G))
        acs_after[G] = c
    vcs_after = {}
    c = 0
    for G in range(NGRP):
        c += len(units_of(G))
        vcs_after[G] = c
    slot_cum = {}
    slot_total = {}
    for s in range(OTN):
        tot = 0
        for G in range(s, NGRP, OTN):
            tot += sum(1 for p in pieces if p[0] == G)
            slot_cum[(s, G)] = tot
        slot_total[s] = tot

    def gt(G):
        return divmod(G, NT)

    # PE increments pes per j-chunk for the ramp groups, per group after
    pes_after = lambda G: (njc * (G + 1) if G <= 1 else 2 * njc + G - 1)

    with ExitStack() as _ctx:
        ec = _ctx.enter_context
        x1s = ec(nc.sbuf_tensor("x1s", [P, KT * NS + XB], dt.float16))
        x2s = ec(nc.sbuf_tensor("x2s", [P, NG * KT * JG], dt.float16))
        erb = ec(nc.sbuf_tensor("erb", [P, M], dt.float16))
        ots = [ec(nc.sbuf_tensor(f"ot{i}", [P, JG], dt.float16)) for i in range(OTN)]
        wrm = ec(nc.sbuf_tensor("wrm", [P, P + NJ], dt.float16))  # uninit junk
        scr = ec(nc.sbuf_tensor("scr", [1, 32], dt.float32))     # table preload
        ps0 = ec(nc.psum_tensor("ps0", [P, JG], dt.float32))
        ps1 = ec(nc.psum_tensor("ps1", [P, JG], dt.float32))
        pss = [ps0, ps1]
        biav = x1s[:, KT * NS:KT * NS + XB].bitcast(dt.float32)  # [P, NT] f32

        s_x1 = ec(nc.semaphore("s_x1"))      # x1 + bias         (16)
        s_x2a = ec(nc.semaphore("s_x2a"))    # x2 g0 j0+j1       (16)
        s_x2b = ec(nc.semaphore("s_x2b"))    # x2 g0 j2+j3       (16)
        s_x2g1 = ec(nc.semaphore("s_x2g1"))  # x2 g1             (16)
        s_x2g2 = ec(nc.semaphore("s_x2g2"))  # x2 g2             (16)
        s_x2g3 = ec(nc.semaphore("s_x2g3"))  # x2 g3             (16)
        ebA = ec(nc.semaphore("ebA"))        # erb g0            (16)
        ebB = ec(nc.semaphore("ebB"))        # erb g1            (16)
        ebC = ec(nc.semaphore("ebC"))        # erb g2+g3         (16)
        pes = ec(nc.semaphore("pes"))
        acs = ec(nc.semaphore("acs"))
        vcs = ec(nc.semaphore("vcs"))
        dps = [ec(nc.semaphore(f"dp{i}")) for i in range(OTN)]
        block = ec(nc.Block())

        ebs_l = [ebA, ebB, ebC, ebC]
        ebs_n = [16, 16, 16, 16]

        def issue_piece(q, pc):
            (G, n, lo, hi, plo, phi, _) = pc
            g, t = gt(G)
            us = units_of(G)
            q.wait_ge(vcs, vcs_after[G] - len(us) + 1 + n)
            q.dma_start(
                outd[t, g, plo:phi, lo:hi],
                ots[G % OTN][plo:phi, lo:hi],
            ).then_inc(dps[G % OTN], 16)

        @block.gpsimd
        def _(gpsimd):
            # SW ring, in global need order: x2 g0 j0+j1 (critical), erb g0
            # (gates mul(G0) -> the G0 output chunk), then even outputs with
            # x2 g1 injected after G2's issue (needed by PE(G8))
            gpsimd.dma_start(x2s[:, 0:JG], x2d[:, 0:JG]).then_inc(s_x2a, 16)
            gpsimd.dma_start(erb[:, 0:JG], rbd[:, 0:JG]).then_inc(ebA, 16)
            gpsimd.dma_start(x2s[:, 2 * JG:4 * JG], x2d[:, 2 * JG:4 * JG]).then_inc(s_x2g1, 16)
            gpsimd.dma_start(x2s[:, 4 * JG:6 * JG], x2d[:, 4 * JG:6 * JG]).then_inc(s_x2g2, 16)
            for pc in pieces:
                if pc[6] == 0:
                    issue_piece(gpsimd, pc)

        @block.sync
        def _(sync):
            # HW ring: x1+bias (critical); late bulk gated behind the
            # criticals; odd outputs; x2 g3 injected mid-stream
            sync.dma_start(x1s[:, :], x1d[:, :]).then_inc(s_x1, 16)
            sync.wait_ge(s_x2a, 16)
            sync.wait_ge(s_x2b, 16)
            sync.dma_start(erb[:, JG:2 * JG], rbd[:, JG:2 * JG]).then_inc(ebB, 16)
            sync.dma_start(erb[:, 2 * JG:4 * JG], rbd[:, 2 * JG:4 * JG]).then_inc(ebC, 16)
            for pc in pieces:
                if pc[6] == 1:
                    issue_piece(sync, pc)
                    if pc[0] == 9:
                        sync.dma_start(x2s[:, 6 * JG:8 * JG],
                                       x2d[:, 6 * JG:8 * JG]).then_inc(s_x2g3, 16)
            for s in range(OTN):
                sync.wait_ge(dps[s], 16 * slot_total[s])

        @block.tensor
        def _(tensor):
            # junk matmuls keep PE busy (HAM warm) until data arrives
            for i in range(WARM):
                mm = tensor.matmul(ps1[:, 0:NJ], wrm[:, 0:P], wrm[:, P:P + NJ],
                                   start=True, stop=True)
                if i > 0:
                    inst = mm.ins
                    inst = inst[0] if isinstance(inst, (list, tuple)) else inst
                    inst.ldweights = False
            # ramp groups G0/G1: j-outer, (k0,k1) per 512-col chunk so ACT
            # drains per chunk while the PE clock is still cold
            for RG in range(2):
                ps = pss[RG % 2]
                for j in range(njc):
                    for k in range(KT):
                        if RG == 0 and j == 0 and k == 0:
                            tensor.wait_ge(s_x1, 16)
                            tensor.wait_ge(s_x2a, 16)
                        if RG == 0 and j == 2 and k == 0:
                            tensor.wait_ge(s_x2b, 16)
                        mm = tensor.matmul(
                            ps[:, j * NJ:(j + 1) * NJ],
                            x1s[:, k * NS + RG * P: k * NS + (RG + 1) * P],
                            x2s[:, j * (KT * NJ) + k * NJ:
                                   j * (KT * NJ) + (k + 1) * NJ],
                            start=(k == 0),
                            stop=(k == KT - 1),
                        )
                    mm.then_inc(pes)
            for G in range(2, NGRP):
                g, t = gt(G)
                if G == NT:
                    tensor.wait_ge(s_x2g1, 16)
                if G == 2 * NT:
                    tensor.wait_ge(s_x2g2, 16)
                if G == 3 * NT:
                    tensor.wait_ge(s_x2g3, 16)
                if G >= 2:
                    tensor.wait_ge(acs, acs_after[G - 2])  # psum half free
                ps = pss[G % 2]
                for k in range(KT):
                    for j in range(njc):
                        mm = tensor.matmul(
                            ps[:, j * NJ:(j + 1) * NJ],
                            x1s[:, k * NS + t * P: k * NS + (t + 1) * P],
                            x2s[:, g * (KT * JG) + j * (KT * NJ) + k * NJ:
                                   g * (KT * JG) + j * (KT * NJ) + (k + 1) * NJ],
                            start=(k == 0),
                            stop=(k == KT - 1),
                        )
                        if j > 0:
                            # same stationary weights as previous matmul:
                            # suppress the redundant LDWEIGHTS
                            inst = mm.ins
                            inst = inst[0] if isinstance(inst, (list, tuple)) else inst
                            inst.ldweights = False
                mm.then_inc(pes)

        @block.scalar
        def _(scalar):
            # touch Exp early (junk SBUF, no DMA dep) so ACT_TABLE_LOAD
            # overlaps engine boot + input DMAs
            scalar.activation(scr[0:1, 16:32], scr[0:1, 0:16], exp_f)
            # third ring (ACT HW DGE): only x2 g0 j2+j3 — the ACT queue
            # must stay clear of gated issue work
            scalar.dma_start(x2s[:, JG:2 * JG], x2d[:, JG:2 * JG]).then_inc(s_x2b, 16)
            scalar.wait_ge(s_x1, 16)   # bias rides the x1 DMA
            for G in range(NGRP):
                g, t = gt(G)
                if G >= OTN:
                    # just-in-time slot-reuse wait: tolerates maximal ring lag
                    sl = G % OTN
                    scalar.wait_ge(dps[sl], 16 * slot_cum[(sl, G - OTN)])
                for n, (lo, hi) in enumerate(act_units(G)):
                    if G <= 1:
                        scalar.wait_ge(pes, pes_after(G) - njc + n + 1)
                    elif n == 0:
                        scalar.wait_ge(pes, pes_after(G))
                    scalar.activation(
                        ots[G % OTN][:, lo:hi],
                        pss[G % 2][:, lo:hi],
                        exp_f,
                        bias=biav[:, t:t + 1],
                        scale=INV_SB,
                    ).then_inc(acs)
            # tail: G31's lower-partition pieces on the (now idle) ACT ring
            for pc in pieces:
                if pc[6] == 2:
                    issue_piece(scalar, pc)

        @block.vector
        def _(vector):
            for G in range(NGRP):
                g, t = gt(G)
                if t == 0:
                    vector.wait_ge(ebs_l[g], ebs_n[g])
                us = units_of(G)
                aus = act_units(G)
                for (lo, hi) in us:
                    need = acs_after[G] - len(aus)
                    for (alo, ahi) in aus:
                        need += 1
                        if ahi >= hi:
                            break
                    vector.wait_ge(acs, need)
                    vector.tensor_mul(ots[G % OTN][:, lo:hi], ots[G % OTN][:, lo:hi],
                                      erb[:, g * JG + lo:g * JG + hi]).then_inc(vcs)

    return nc


def _prep(x1, x2, alpha_raw, variance_raw):
    x1 = np.ascontiguousarray(np.asarray(x1, dtype=np.float32))
    x2 = np.ascontiguousarray(np.asarray(x2, dtype=np.float32))
    ar = np.asarray(alpha_raw, dtype=np.float64).reshape(-1)
    vr = np.asarray(variance_raw, dtype=np.float64).reshape(-1)

    a2 = ar * ar
    e = np.exp(a2 - a2.max())
    alpha = e / e.sum()                                   # (D,) f64
    var = float(vr[0]) ** 2
    if var > 0.0:
        logvar, post = math.log(var), None
    else:
        logvar, post = 0.0, var

    b = (SCALE_B * alpha[None, :]) * x2.astype(np.float64)  # (M, D)
    x2tm = b.T.reshape(KT, P, M).astype(_F16)             # [k, p, col]
    # device layout: col = g*(KT*JG) + jc*(KT*NJ) + k*NJ + jj
    njc = JG // NJ
    x2c = np.ascontiguousarray(
        x2tm.reshape(KT, P, NG, njc, NJ)
        .transpose(1, 2, 3, 0, 4)
        .reshape(P, NG * KT * JG))
    x1tm = x1.T.reshape(KT, P, N).astype(_F16)            # [k, p, row]

    ra = (x1.astype(np.float64) ** 2) @ alpha             # (N,)
    rb = (x2.astype(np.float64) ** 2) @ alpha             # (M,)
    bia = (-0.5 * ra + logvar).astype(np.float32)         # (N,)
    rbrow = np.exp(-0.5 * rb).astype(_F16).reshape(1, M)
    rbd = np.ascontiguousarray(np.broadcast_to(rbrow, (P, M)))

    in_maps = []
    for c in range(NCORES):
        sl = slice(c * NS, (c + 1) * NS)
        bia2 = np.ascontiguousarray(
            bia[sl].reshape(NT, P).T.astype(np.float32))   # [p, t]
        bia16 = bia2.view(np.float16)                      # [p, 2*NT] raw bytes
        x1c = np.ascontiguousarray(
            np.concatenate([x1tm[0][:, sl], x1tm[1][:, sl], bia16], axis=1))
        in_maps.append({
            "x1d": x1c,
            "x2d": x2c,
            "rbd": rbd,
        })
    return in_maps, post


def _untile(blk):
    # [NT, NG, P, JG] -> [NS, M]
    a = np.asarray(blk).reshape(NT, NG, P, JG)
    return a.transpose(0, 2, 1, 3).reshape(NS, M)


def _run(in_maps, trace=False):
    global _compiled
    from concourse.bass_utils import run_bass_kernel_spmd

    if _compiled is None:
        _compiled = _build()
    return run_bass_kernel_spmd(
        _compiled, in_maps, core_ids=list(range(NCORES)), trace=trace
    )


def kernel(x1, x2, alpha_raw, variance_raw):
    in_maps, post = _prep(x1, x2, alpha_raw, variance_raw)
    res = _run(in_maps)
    full = np.concatenate(
        [_untile(res.results[c]["out"]).astype(np.float32) for c in range(NCORES)],
        axis=0)
    if post is not None:
        full = (full * post).astype(np.float32)
    return full
